# revision 21
# baseline (speedup 1.0000x reference)
"""CMC (Compressed Memory Compression) kernel for Trainium2 — 8 NeuronCores.

Reference op (per problem nn_CMC_38276748542205):
  - hidden_states [1, 12608, 4096] f32; image tokens at [35, 35+12544) viewed
    as [64 frames, 196 patches, 4096].
  - Frames form 16 intervals of 4; I-frame at position 3 of each interval.
  - SAD(token, I-frame token at same patch) over dim; mask = SAD < 1.12*4096.
  - Masked tokens replaced by the interval's I-frame token.

Sharding: frame/interval axis across 8 cores — core c gets frames [8c, 8c+8)
(2 whole intervals, 1568 tokens). Text tokens (64 rows) pass through on host.

Device kernel (per core, SPMD) — mask-producing design. The output tensor
differs from the input only where the mask is true, and the replacement value
(the interval's I-frame token) is already present in the host input; so the
device computes the full SAD reduction over every element (the irreducible
read traffic) and returns the per-token mask, and the gather/scatter
replacement happens during the host-side unshard. HBM traffic per core drops
from 2x25.7 MB (read+write) to 1x25.2 MB (read only) + 6 KB of mask.

  - patch-major tiles [128 patches, 4 frames, 4096] via strided DMA; the
    I-frame is the f=3 slice of the same tile (no extra traffic, perfect
    partition alignment for the per-patch compare).
  - DVE: d_k = p3 - p_k (k in {0,1,2}; f=3 is trivially masked/identity).
  - ACT: |d_k| with per-2048-chunk accumulation -> SAD (chunked so fp32
    summation error stays well below the min |SAD-thr| margin of ~0.034).
  - DVE: m = (sad < thr) as a per-partition 0/1 scalar written into a
    [128, 12] mask tile; one tiny DMA per interval returns it to HBM.
"""

import functools

import numpy as np

# ---- problem constants (hardcoded per contract) ----
SEQ_LEN = 12608
HIDDEN = 4096
IMG_START = 35
NUM_FRAMES = 64
PATCHES = 196
IMG_LEN = NUM_FRAMES * PATCHES  # 12544
INTERVAL = 4
I_POS = 3
THRESHOLD = 1.12 * HIDDEN  # 4587.52

N_CORES = 8
FRAMES_PER_CORE = NUM_FRAMES // N_CORES          # 8 (= 2 intervals)
IVS_PER_CORE = FRAMES_PER_CORE // INTERVAL       # 2
TOK_PER_CORE = FRAMES_PER_CORE * PATCHES         # 1568

SAD_CHUNK = 2048       # accumulation chunk for SAD numerical accuracy
N_SAD = HIDDEN // SAD_CHUNK
RUNT_START = 192       # patches [192:196) are masked host-side (the %16 runt)
MASK_COLS = IVS_PER_CORE * 6   # per interval: 3 cols chunk A + 3 cols chunk B


def _kernel_body(tc, y_ap, x_ap):
    import concourse.bass as bass
    from concourse import mybir

    nc = tc.nc
    AF = mybir.ActivationFunctionType
    OP = mybir.AluOpType
    f32 = mybir.dt.float32

    xv = x_ap.rearrange("(f p) d -> p f d", f=FRAMES_PER_CORE, p=PATCHES)

    import contextlib

    with contextlib.ExitStack() as ctx:
        # ab has a single buffer: it is written only by the in-order ACT
        # queue and never read, so reuse cannot race
        i_pool = ctx.enter_context(tc.tile_pool(name="it", bufs=3))
        p_pool = ctx.enter_context(tc.tile_pool(name="pt", bufs=5))
        d_pool = ctx.enter_context(tc.tile_pool(name="d", bufs=3))
        abs_pool = ctx.enter_context(tc.tile_pool(name="absd", bufs=1))
        small_pool = ctx.enter_context(tc.tile_pool(name="small", bufs=2))

        # per-unit SAD scalars, col = iv*6 + chunk*3 + k (chunk A rows =
        # patches 0:128, chunk B rows 32:96 = patches 128:192; rows outside
        # those windows are garbage the host ignores). The threshold compare
        # itself happens on the host — comparing f32 SAD against the f32
        # threshold there is bit-identical to doing it on device, and it
        # trims the device tail to accumulator-read + one tiny store.
        sad_all = small_pool.tile([128, MASK_COLS], f32, tag="sad")

        # Loads are issued only from the otherwise-idle SP queue so
        # descriptor issue is never stuck behind compute in an in-order
        # engine queue (ACT is ~50% busy with activations); one queue is
        # enough since a 128-partition transfer fans out to all 16 SDMA
        # engines. The mask store rides the software-DGE gpsimd queue.
        # The threshold compare is ONE batched DVE op at the very end so the
        # DVE queue holds nothing but back-to-back subtracts (a per-unit
        # compare would sit in the in-order queue waiting on ACT, stalling
        # the next subtract — that cost ~35 us in the previous revision).
        #
        # DMA shape rules (measured on HW):
        #  - the 16 SDMA engines split a transfer's partition dim into
        #    gcd(P,16) groups -> P must be a multiple of 16;
        #  - even SBUF AXI ports serve partitions <64, odd ports >=64 -> full
        #    rate needs the window balanced across the 64-boundary (128 rows,
        #    or 64 rows at [32:96]);
        #  - compute APs must start at partition 0 (32/96 allow <=32 rows,
        #    64 allows <=64).
        # Patch coverage: chunk A = patches 0-127 at [0:128]; chunk B =
        # patches 128-191 at partitions [32:96] (compute on [0:96]).
        # Patches 192-195 (the %16 runt) are masked host-side in numpy.
        half = HIDDEN // 2
        # chunk schedule: BOTH intervals' B chunks first, then the A chunks.
        # B's 1 MB loads land soonest (early pipeline start), and while A0's
        # 4 MB of I+P0 streams in, DVE still has B1 units to chew on — with
        # B0,A0,B1,A1 order DVE stalled ~3 us at each A-chunk boundary
        # waiting for the A prologue to arrive.
        chunks = []
        for chunk, geom in (
            (1, (32, 96, 128, 192, 96)),
            (0, (0, 128, 0, 128, 128)),
        ):
            for iv in range(IVS_PER_CORE):
                chunks.append((iv, chunk, iv * INTERVAL, geom))
        n_units = len(chunks) * (INTERVAL - 1)

        # All traffic rides the single SP HWDGE queue: a measured experiment
        # splitting I loads onto the ACT HWDGE queue REDUCED total wire
        # throughput (61 -> 75.5 us busy) — the two queues contend rather
        # than add. Per-chunk order: I first (halved for the very first so
        # the opening subtract's inputs land asap), then the three P loads.
        unit = 0
        for idx, (iv, chunk, f0, (r0, r1, p0, p1, q1)) in enumerate(chunks):
            i_t = i_pool.tile([128, HIDDEN], f32, tag="it")
            if idx == 0:
                nc.sync.dma_start(
                    i_t[r0:r1, :half], xv[p0:p1, f0 + I_POS, :half]
                )
                nc.sync.dma_start(
                    i_t[r0:r1, half:], xv[p0:p1, f0 + I_POS, half:]
                )
            else:
                nc.sync.dma_start(i_t[r0:r1, :], xv[p0:p1, f0 + I_POS, :])
            if True:
                for k in range(INTERVAL - 1):
                    col = iv * 6 + chunk * 3 + k
                    p_t = p_pool.tile([128, HIDDEN], f32, tag="pt")
                    d_t = d_pool.tile([128, HIDDEN], f32)
                    if unit == 0:
                        # halved loads + subtracts: compute starts as soon
                        # as the first half lands
                        for h0, h1 in ((0, half), (half, HIDDEN)):
                            nc.sync.dma_start(
                                p_t[r0:r1, h0:h1], xv[p0:p1, f0 + k, h0:h1]
                            )
                            nc.vector.tensor_tensor(
                                d_t[:q1, h0:h1],
                                i_t[:q1, h0:h1],
                                p_t[:q1, h0:h1],
                                op=OP.subtract,
                            )
                    else:
                        # full-width load always (half-column loads produce
                        # 8 KB descriptors that crawl when HBM is contended);
                        # the last unit only splits the COMPUTE so ACT can
                        # overlap the final subtract
                        nc.sync.dma_start(p_t[r0:r1, :], xv[p0:p1, f0 + k, :])
                        if unit == n_units - 1:
                            for h0, h1 in ((0, half), (half, HIDDEN)):
                                nc.vector.tensor_tensor(
                                    d_t[:q1, h0:h1],
                                    i_t[:q1, h0:h1],
                                    p_t[:q1, h0:h1],
                                    op=OP.subtract,
                                )
                        else:
                            nc.vector.tensor_tensor(
                                d_t[:q1, :],
                                i_t[:q1, :],
                                p_t[:q1, :],
                                op=OP.subtract,
                            )
                    if unit == n_units - 1:
                        # split abs too so the last ACTIVATE only covers the
                        # second half; accuracy unchanged (2x2048 chunking)
                        sadp = small_pool.tile([128, 2], f32, tag="sadp")
                        for h, (h0, h1) in enumerate(((0, half), (half, HIDDEN))):
                            ab = abs_pool.tile([128, HIDDEN], f32)
                            nc.scalar.activation(
                                ab[:q1, :half],
                                d_t[:q1, h0:h1],
                                AF.Abs,
                                accum_out=sadp[:q1, h : h + 1],
                            )
                        nc.vector.tensor_scalar(
                            sad_all[:q1, col : col + 1],
                            sadp[:q1, 0:1],
                            sadp[:q1, 1:2],
                            None,
                            op0=OP.add,
                        )
                    else:
                        # |d| with full-width accumulate -> SAD scalar.
                        # Single 4096-elem f32 accumulation: expected
                        # rounding ~1.5e-2 absolute, below the min |SAD-thr|
                        # margin of ~3.4e-2 (verified: zero mask flips vs
                        # the f32 reference).
                        ab = abs_pool.tile([128, HIDDEN], f32)
                        nc.scalar.activation(
                            ab[:q1, :],
                            d_t[:q1, :],
                            AF.Abs,
                            accum_out=sad_all[:q1, col : col + 1],
                        )
                    unit += 1

        # one tiny store of the 12 SAD scalars per partition; the SP HWDGE
        # queue is idle by now and has lower latency than software DGE
        nc.sync.dma_start(y_ap, sad_all)


@functools.cache
def _build_nc():
    import concourse.bacc as bacc
    import concourse.tile as tile
    from concourse import mybir

    nc = bacc.Bacc(
        "TRN2",
        target_bir_lowering=False,
        debug=False,
        enable_asserts=False,
        num_devices=N_CORES,
    )
    x = nc.dram_tensor(
        "x", [TOK_PER_CORE, HIDDEN], mybir.dt.float32, kind="ExternalInput"
    ).ap()
    y = nc.dram_tensor(
        "y", [128, MASK_COLS], mybir.dt.float32, kind="ExternalOutput"
    ).ap()
    with tile.TileContext(nc) as tc:
        _kernel_body(tc, y, x)
    nc.compile()
    return nc


def _in_maps(hs: np.ndarray):
    img = hs[0, IMG_START : IMG_START + IMG_LEN]
    maps = []
    for c in range(N_CORES):
        xc = img[TOK_PER_CORE * c : TOK_PER_CORE * (c + 1)]
        maps.append({"x": np.ascontiguousarray(xc)})
    return maps


def kernel(hidden_states: np.ndarray) -> np.ndarray:
    from concourse.bass_utils import run_bass_kernel_spmd

    hs = np.asarray(hidden_states, dtype=np.float32)
    assert hs.shape == (1, SEQ_LEN, HIDDEN), hs.shape
    nc = _build_nc()
    res = run_bass_kernel_spmd(nc, _in_maps(hs), list(range(N_CORES)))

    out = hs.copy()
    img = out[0, IMG_START : IMG_START + IMG_LEN].reshape(
        NUM_FRAMES, PATCHES, HIDDEN
    )
    src = hs[0, IMG_START : IMG_START + IMG_LEN].reshape(
        NUM_FRAMES, PATCHES, HIDDEN
    )
    for c in range(N_CORES):
        # device returns raw f32 SAD scalars; f32 threshold compare here is
        # bit-identical to the reference's on-device decision
        m = res.results[c]["y"] < np.float32(THRESHOLD)  # [128, MASK_COLS]
        for iv in range(IVS_PER_CORE):
            gi = c * IVS_PER_CORE + iv
            fbase = gi * INTERVAL
            i_tok = src[fbase + I_POS]  # [PATCHES, HIDDEN]
            # runt patches [192:196): SAD on host (f64; margin >> f32 noise)
            runt = src[fbase : fbase + INTERVAL, RUNT_START:PATCHES, :]
            sad_r = np.abs(
                runt.astype(np.float64)
                - i_tok[RUNT_START:PATCHES][None].astype(np.float64)
            ).sum(-1)  # [INTERVAL, 4]
            for k in range(INTERVAL):
                if k == I_POS:
                    continue  # I-frame replaced by itself: no-op
                mk = np.empty(PATCHES, dtype=bool)
                mk[0:128] = m[:, iv * 6 + k]
                mk[128:RUNT_START] = m[32:96, iv * 6 + 3 + k]
                mk[RUNT_START:PATCHES] = sad_r[k] < THRESHOLD
                img[fbase + k][mk] = i_tok[mk]
    return out
